# revision 1
# baseline (speedup 1.0000x reference)
"""Trainium2 Bass kernel for nn_ConvolutionalCapsules.

Sharding: core c (of 8) owns output-capsule nout=c. Each core runs the p4 group
conv restricted to its 64 output channels (16 dout x 4 rot) over all 32 images
(B*Nin), then LayerNorm + degree-score routing + squash for its nout.

Conv: 3x3 conv as shifted matmuls from a zero-padded SBUF image (34x34 rows).
Partitions 0-63 hold the padded image (copy A), partitions 64-127 hold the same
image shifted by one padded row (copy B), so one K=128 matmul covers two filter
taps: (0,kx) on A plus (1,kx) on B at base offset kx. Row-2 taps run as K=64
matmuls on copy A. 6 matmuls accumulate one PSUM tile of 512 positions.

Routing runs in a transposed layout (positions on partitions, (i,d,g) on the
free axis) produced by PE transpose-mode, so every reduction (d, i, g) is a
free-axis tensor_reduce.
"""

import numpy as np
from contextlib import ExitStack

import concourse.bass as bass
import concourse.tile as tile
from concourse import mybir
from concourse.bass_utils import run_bass_kernel_spmd

F32 = mybir.dt.float32
F32R = mybir.dt.float32r
AF = mybir.ActivationFunctionType
OP = mybir.AluOpType
AX = mybir.AxisListType

MM_DT = F32R  # float32r: full-rate PE at ~tf32 precision; set F32 for exact

_ENGINES = {
    mybir.EngineType.PE,
    mybir.EngineType.Activation,
    mybir.EngineType.Pool,
    mybir.EngineType.DVE,
    mybir.EngineType.SP,
}


def _split_sync_waits(nc):
    """This walrus build accepts a single embedded sync-wait per instruction;
    hoist extras onto preceding NoOps on the same engine (ge-imm waits commute)."""
    for f in nc.m.functions:
        for bb in f.blocks:
            newl = []
            changed = False
            for inst in list(bb.instructions):
                si = inst.sync_info
                waits = list(si.on_wait) if si and si.on_wait else []
                if len(waits) > 1 and inst.engine in _ENGINES:
                    changed = True
                    for k, w in enumerate(waits[:-1]):
                        newl.append(
                            mybir.InstNoOp(
                                name=f"{inst.name}-ws{k}",
                                ins=[],
                                outs=[],
                                engine=inst.engine,
                                sync_info=mybir.SyncInfo(on_wait=[w], on_update=[]),
                            )
                        )
                    si.on_wait = waits[-1:]
                    inst.sync_info = si
                newl.append(inst)
            if changed:
                bb.instructions = newl


def build_program(apply_bias=False, apply_gb=False):
    nc = bass.Bass(trn_type="TRN2")
    caps = nc.dram_tensor("caps", [4, 8, 16, 4, 32, 32], MM_DT, kind="ExternalInput")
    w = nc.dram_tensor("w", [128, 384], MM_DT, kind="ExternalInput")
    ident = nc.dram_tensor("ident", [128, 128], F32, kind="ExternalInput")
    zer = nc.dram_tensor("zer", [1, 1164], MM_DT, kind="ExternalInput")
    if apply_bias:
        cb = nc.dram_tensor("cb", [64, 1], F32, kind="ExternalInput")
    if apply_gb:
        gam = nc.dram_tensor("gam", [1, 16], F32, kind="ExternalInput")
        bet = nc.dram_tensor("bet", [1, 16], F32, kind="ExternalInput")
    out = nc.dram_tensor("out", [4, 16, 4, 32, 32], F32, kind="ExternalOutput")

    caps_r = caps.ap().rearrange("b n d g h w -> (b n) (d g) h w")  # [32,64,32,32]
    out_r = out.ap().rearrange("b d g h w -> b (h w) d g")  # [4,1024,16,4]

    XW = 1164  # padded 34x34 image (1156) + slack so 16-row AP views stay in-bounds

    with tile.TileContext(nc) as tc:
        with ExitStack() as ctx:
            singles = ctx.enter_context(tc.tile_pool(name="singles", bufs=1))
            ps_conv = ctx.enter_context(tc.tile_pool(name="ps_conv", bufs=4, space="PSUM"))
            ps_tr = ctx.enter_context(tc.tile_pool(name="ps_tr", bufs=3, space="PSUM"))
            tpool = ctx.enter_context(tc.tile_pool(name="tpool", bufs=3))
            rbig = ctx.enter_context(tc.tile_pool(name="rbig", bufs=3))
            sm = ctx.enter_context(tc.tile_pool(name="sm", bufs=3))
            vout = ctx.enter_context(tc.tile_pool(name="vout", bufs=2))

            w_sb = singles.tile([128, 384], MM_DT, tag="w")
            nc.sync.dma_start(out=w_sb[:], in_=w.ap())
            id_sb = singles.tile([128, 128], F32, tag="ident")
            nc.sync.dma_start(out=id_sb[:], in_=ident.ap())
            if apply_bias:
                cb_sb = singles.tile([64, 1], F32, tag="cb")
                nc.sync.dma_start(out=cb_sb[:], in_=cb.ap())
            if apply_gb:
                gam_sb = singles.tile([128, 16], F32, tag="gam")
                nc.sync.dma_start(out=gam_sb[:], in_=gam.ap().partition_broadcast(128))
                bet_sb = singles.tile([128, 16], F32, tag="bet")
                nc.sync.dma_start(out=bet_sb[:], in_=bet.ap().partition_broadcast(128))

            eps5 = singles.tile([128, 1], F32, tag="eps5")
            nc.vector.memset(eps5[:], 1e-5)
            eps16 = singles.tile([128, 1], F32, tag="eps16")
            nc.vector.memset(eps16[:], 1e-16)

            xpads = []
            for ix in range(3):
                xp = singles.tile([128, XW], MM_DT, tag=f"xpad{ix}", name=f"xpad{ix}")
                nc.sync.dma_start(out=xp[:], in_=zer.ap().partition_broadcast(128))
                xpads.append(xp)

            u_sb = [
                [singles.tile([128, 1024], F32, tag=f"u{b}_{p}", name=f"u{b}_{p}") for p in range(4)]
                for b in range(4)
            ]

            def hview(ap_flat, o, rows):
                """[P, rows, 32] window at flat offset o with padded row stride 34."""
                return ap_flat[:, o: o + rows * 34].rearrange(
                    "c (h w) -> c h w", w=34
                )[:, :, 0:32]

            for b in range(4):
                # ---- conv for the 8 images of this batch ----
                for n in range(8):
                    img = b * 8 + n
                    xp = xpads[img % 3]
                    src = caps_r[img]  # [64,32,32]
                    dstA = hview(xp[0:64], 35, 32)
                    dstB = hview(xp[64:128], 1, 32)
                    nc.sync.dma_start(out=dstA, in_=src)
                    nc.sync.dma_start(out=dstB, in_=src)
                    pair, half = n // 2, n % 2
                    for chh in range(2):
                        ps = ps_conv.tile([64, 512], F32, tag="ps")
                        base = chh * 16 * 34
                        for kx in range(3):
                            rhs = hview(xp, base + kx, 16)
                            nc.tensor.matmul(
                                ps[:],
                                lhsT=w_sb[:, kx * 64:(kx + 1) * 64],
                                rhs=rhs,
                                start=(kx == 0),
                                stop=False,
                            )
                        for kx in range(3):
                            rhs = hview(xp[0:64], base + 68 + kx, 16)
                            nc.tensor.matmul(
                                ps[:],
                                lhsT=w_sb[0:64, (3 + kx) * 64:(4 + kx) * 64],
                                rhs=rhs,
                                start=False,
                                stop=(kx == 2),
                            )
                        dst = u_sb[b][pair][half * 64:(half + 1) * 64, chh * 512:(chh + 1) * 512]
                        if apply_bias:
                            nc.scalar.activation(dst, ps[:], AF.Identity, bias=cb_sb[:], scale=1.0)
                        else:
                            nc.scalar.activation(dst, ps[:], AF.Copy)

                # ---- transpose + routing, two steps of 4 position-blocks ----
                for sh in range(2):
                    T = tpool.tile([128, 2048], F32, tag="T")
                    for bq in range(4):
                        blk = sh * 4 + bq
                        pst = ps_tr.tile([128, 512], F32, tag="pst")
                        for p in range(4):
                            nc.tensor.transpose(
                                out=pst[:, p * 128:(p + 1) * 128],
                                in_=u_sb[b][p][:, blk * 128:(blk + 1) * 128],
                                identity=id_sb[:],
                            )
                        nc.scalar.activation(T[:, bq * 512:(bq + 1) * 512], pst[:], AF.Copy)

                    # views: col = k*512 + i*64 + d*4 + g
                    T5 = T.rearrange("p (k i d g) -> p k i d g", k=4, i=8, d=16)

                    mu = sm.tile([128, 128], F32, tag="mu")
                    mu4 = mu.rearrange("p (k i g) -> p k i g", k=4, i=8)
                    nc.vector.reduce_sum(mu4, T5.transpose((0, 1, 2, 4, 3)), AX.X)

                    sq = rbig.tile([128, 2048], F32, tag="scratch")
                    nc.scalar.activation(sq[:], T[:], AF.Square)
                    sq5 = sq.rearrange("p (k i d g) -> p k i d g", k=4, i=8, d=16)
                    msq = sm.tile([128, 128], F32, tag="msq")
                    msq4 = msq.rearrange("p (k i g) -> p k i g", k=4, i=8)
                    nc.vector.reduce_sum(msq4, sq5.transpose((0, 1, 2, 4, 3)), AX.X)

                    m1 = sm.tile([128, 128], F32, tag="m1")
                    nc.vector.tensor_scalar_mul(out=m1[:], in0=mu[:], scalar1=1.0 / 16.0)
                    var = sm.tile([128, 128], F32, tag="var")
                    nc.vector.tensor_tensor(out=var[:], in0=m1[:], in1=m1[:], op=OP.mult)
                    nc.vector.scalar_tensor_tensor(
                        out=var[:], in0=msq[:], scalar=1.0 / 16.0, in1=var[:],
                        op0=OP.mult, op1=OP.subtract,
                    )
                    rstd = sm.tile([128, 128], F32, tag="rstd")
                    nc.scalar.activation(rstd[:], var[:], AF.Sqrt, bias=eps5[:])
                    nc.vector.reciprocal(rstd[:], rstd[:])
                    n2 = sm.tile([128, 128], F32, tag="n2")
                    nc.vector.tensor_tensor(out=n2[:], in0=m1[:], in1=rstd[:], op=OP.mult)

                    def bc_kig(t):  # [128,128] (k,i,g) -> [p,k,i,d,g]
                        return (
                            t.rearrange("p (k i g) -> p k i g", k=4, i=8)
                            .unsqueeze(3)
                            .broadcast_to((128, 4, 8, 16, 4))
                        )

                    up = rbig.tile([128, 2048], F32, tag="up")
                    up5 = up.rearrange("p (k i d g) -> p k i d g", k=4, i=8, d=16)
                    nc.vector.tensor_tensor(out=up5, in0=T5, in1=bc_kig(rstd), op=OP.mult)
                    nc.vector.tensor_tensor(out=up5, in0=up5, in1=bc_kig(n2), op=OP.subtract)
                    if apply_gb:
                        gb = gam_sb[:].unsqueeze(1).unsqueeze(2).unsqueeze(4).broadcast_to((128, 4, 8, 16, 4))
                        bb_ = bet_sb[:].unsqueeze(1).unsqueeze(2).unsqueeze(4).broadcast_to((128, 4, 8, 16, 4))
                        nc.vector.tensor_tensor(out=up5, in0=up5, in1=gb, op=OP.mult)
                        nc.vector.tensor_tensor(out=up5, in0=up5, in1=bb_, op=OP.add)

                    S = sm.tile([128, 256], F32, tag="S")
                    S4 = S.rearrange("p (k d g) -> p k d g", k=4, d=16)
                    nc.vector.reduce_sum(S4, up5.transpose((0, 1, 3, 4, 2)), AX.X)

                    P = rbig.tile([128, 2048], F32, tag="scratch")
                    P5 = P.rearrange("p (k i d g) -> p k i d g", k=4, i=8, d=16)
                    S_bc = S4.unsqueeze(2).broadcast_to((128, 4, 8, 16, 4))
                    nc.vector.tensor_tensor(out=P5, in0=up5, in1=S_bc, op=OP.mult)
                    dot = sm.tile([128, 128], F32, tag="dot")
                    dot4 = dot.rearrange("p (k i g) -> p k i g", k=4, i=8)
                    nc.vector.reduce_sum(dot4, P5.transpose((0, 1, 2, 4, 3)), AX.X)

                    ns = sm.tile([128, 128], F32, tag="ns")
                    nc.vector.tensor_tensor(out=ns[:], in0=rstd[:], in1=rstd[:], op=OP.mult)
                    nc.vector.scalar_tensor_tensor(
                        out=ns[:], in0=var[:], scalar=16.0, in1=ns[:],
                        op0=OP.mult, op1=OP.mult,
                    )
                    nc.vector.reciprocal(ns[:], ns[:])
                    rr = sm.tile([128, 128], F32, tag="rr")
                    nc.vector.tensor_tensor(out=rr[:], in0=dot[:], in1=ns[:], op=OP.mult)

                    rr4 = rr.rearrange("p (k i g) -> p k i g", k=4, i=8)
                    mx = sm.tile([128, 16], F32, tag="mx")
                    mx3 = mx.rearrange("p (k g) -> p k g", k=4)
                    nc.vector.reduce_max(mx3, rr4.transpose((0, 1, 3, 2)), AX.X)
                    es = sm.tile([128, 128], F32, tag="es")
                    es4 = es.rearrange("p (k i g) -> p k i g", k=4, i=8)
                    mx_bc = mx3.unsqueeze(2).broadcast_to((128, 4, 8, 4))
                    nc.vector.tensor_tensor(out=es4, in0=rr4, in1=mx_bc, op=OP.subtract)
                    nc.scalar.activation(es[:], es[:], AF.Exp)
                    Z = sm.tile([128, 16], F32, tag="Z")
                    Z3 = Z.rearrange("p (k g) -> p k g", k=4)
                    nc.vector.reduce_sum(Z3, es4.transpose((0, 1, 3, 2)), AX.X)
                    nc.vector.reciprocal(Z[:], Z[:])
                    sc = sm.tile([128, 128], F32, tag="sc")
                    sc4 = sc.rearrange("p (k i g) -> p k i g", k=4, i=8)
                    Z_bc = Z3.unsqueeze(2).broadcast_to((128, 4, 8, 4))
                    nc.vector.tensor_tensor(out=sc4, in0=es4, in1=Z_bc, op=OP.mult)

                    nc.vector.tensor_tensor(out=P5, in0=up5, in1=bc_kig(sc), op=OP.mult)
                    s_t = sm.tile([128, 256], F32, tag="s")
                    s4 = s_t.rearrange("p (k d g) -> p k d g", k=4, d=16)
                    nc.vector.reduce_sum(s4, P5.transpose((0, 1, 3, 4, 2)), AX.X)

                    ssq = sm.tile([128, 256], F32, tag="ssq")
                    nc.scalar.activation(ssq[:], s_t[:], AF.Square)
                    nsq = sm.tile([128, 64], F32, tag="nsq")
                    nsq3 = nsq.rearrange("p (k d) -> p k d", k=4)
                    nc.vector.reduce_sum(nsq3, ssq.rearrange("p (k d g) -> p k d g", k=4, d=16), AX.X)
                    sq1 = sm.tile([128, 64], F32, tag="sq1")
                    nc.scalar.activation(sq1[:], nsq[:], AF.Sqrt, bias=eps16[:])
                    nc.vector.scalar_tensor_tensor(
                        out=sq1[:], in0=nsq[:], scalar=1.0, in1=sq1[:],
                        op0=OP.add, op1=OP.mult,
                    )
                    nc.vector.reciprocal(sq1[:], sq1[:])
                    f = sm.tile([128, 64], F32, tag="f")
                    nc.vector.tensor_tensor(out=f[:], in0=nsq[:], in1=sq1[:], op=OP.mult)

                    v = vout.tile([128, 256], F32, tag="v")
                    v4 = v.rearrange("p (k d g) -> p k d g", k=4, d=16)
                    f_bc = f.rearrange("p (k d) -> p k d", k=4).unsqueeze(3).broadcast_to((128, 4, 16, 4))
                    nc.vector.tensor_tensor(out=v4, in0=s4, in1=f_bc, op=OP.mult)

                    dstv = out_r[b].rearrange("(kk p) d g -> p kk d g", p=128)
                    for kk in range(4):
                        nc.sync.dma_start(
                            out=dstv[:, sh * 4 + kk, :, :], in_=v4[:, kk, :, :]
                        )

    _split_sync_waits(nc)
    return nc


def _pack_weights(conv_w):
    w = np.asarray(conv_w, np.float32)
    wt = np.stack(
        [np.roll(np.rot90(w, k=r, axes=(3, 4)), r, axis=2) for r in range(4)], axis=1
    )
    W512 = np.ascontiguousarray(wt.reshape(512, 64, 3, 3), dtype=np.float32)
    packs = []
    for c in range(8):
        Wc = W512[64 * c: 64 * c + 64]
        w_pack = np.zeros((128, 6, 64), np.float32)
        for kx in range(3):
            w_pack[0:64, kx] = Wc[:, :, 0, kx].T
            w_pack[64:128, kx] = Wc[:, :, 1, kx].T
            w_pack[0:64, 3 + kx] = Wc[:, :, 2, kx].T
        packs.append(np.ascontiguousarray(w_pack.reshape(128, 384)))
    return packs


_CACHE = {}


def kernel(capsules, conv_w, conv_b, ln_gamma, ln_beta):
    capsules = np.ascontiguousarray(np.asarray(capsules, np.float32))
    conv_b = np.asarray(conv_b, np.float32)
    ln_gamma = np.asarray(ln_gamma, np.float32)
    ln_beta = np.asarray(ln_beta, np.float32)
    apply_bias = bool(np.any(conv_b))
    apply_gb = bool(np.any(ln_gamma != 1.0) or np.any(ln_beta != 0.0))

    key = (apply_bias, apply_gb)
    if key not in _CACHE:
        _CACHE[key] = build_program(apply_bias=apply_bias, apply_gb=apply_gb)
    nc = _CACHE[key]

    packs = _pack_weights(conv_w)
    ident = np.eye(128, dtype=np.float32)
    in_maps = []
    for c in range(8):
        m = {"caps": capsules, "w": packs[c], "ident": ident,
             "zer": np.zeros((1, 1164), np.float32)}
        if apply_bias:
            b_loc = np.repeat(conv_b[c * 16:(c + 1) * 16], 4)  # partition = d*4+g
            m["cb"] = np.ascontiguousarray(b_loc.reshape(64, 1))
        if apply_gb:
            m["gam"] = np.ascontiguousarray(ln_gamma.reshape(1, 16))
            m["bet"] = np.ascontiguousarray(ln_beta.reshape(1, 16))
        in_maps.append(m)

    res = run_bass_kernel_spmd(nc, in_maps, core_ids=list(range(8)), trace=False)
    out = np.stack([res.results[c]["out"] for c in range(8)], axis=1)
    return out.astype(np.float32)



# revision 11
# speedup vs baseline: 2.0804x; 2.0804x over previous
"""Trainium2 Bass kernel for nn_ConvolutionalCapsules.

Sharding: 8 cores = 4 nout-pairs x 2 batch-halves. Core (p, h) runs the p4
group conv for output channels of nouts {2p, 2p+1} (M=128) over the 16 images
(b in {2h, 2h+1}) x (nin 0..7), then LayerNorm + degree-score routing + squash
for those (2 b, 2 nout) blocks.

Conv: 3x3 conv as shifted matmuls from a host-prepadded doubled image in bf16:
partitions 0-63 hold the zero-padded 34x34 image, partitions 64-127 the same
image shifted up one padded row, so one K=128 matmul covers taps (0,kx)+(1,kx);
row-2 taps run as K=64 matmuls on the first half. 6 matmuls accumulate one
[128, 512] PSUM tile (128 out channels x 512 positions).

Output channels are ordered (n', d, g); broadcast operands keep a real
stride-1 innermost dim (g) so DVE bf16 TTs stay in the 2x perf mode, with
merged views keeping every AP at <=4 free dims.

Routing: LN statistics (sum over d) are computed by PE mask-matmuls straight
from the channel-major conv output (lhsT = u block, rhs = 8-col group mask,
~free on PE). Elementwise work runs pos-major in bf16 on DVE (2x mode) on
PE-transposed blocks; sums over the 8 input capsules use PE identity-matmul
PSUM accumulation. The LN centering is folded algebraically:
  P = T*rstd,  up = P - n2   (n2 = mu*rstd per site)
  S = sum_i P - (sum_i n2);  dot_i = sum_d P*S - n2_i * sum_d S
  s = sum_i sc_i*P - sum_i sc_i*n2_i
so no explicit centered tensor is materialized.
"""

import numpy as np
from contextlib import ExitStack

import concourse.bass as bass
import concourse.tile as tile
from concourse import mybir
from concourse.bass_utils import run_bass_kernel_spmd

F32 = mybir.dt.float32
BF16 = mybir.dt.bfloat16
AF = mybir.ActivationFunctionType
OP = mybir.AluOpType
AX = mybir.AxisListType

XW = 1164  # padded 34x34 image (1156) + slack so 16-row AP views stay in-bounds

_ENGINES = {
    mybir.EngineType.PE,
    mybir.EngineType.Activation,
    mybir.EngineType.Pool,
    mybir.EngineType.DVE,
    mybir.EngineType.SP,
}


def _split_sync_waits(nc):
    """This walrus build accepts a single embedded sync-wait per instruction;
    hoist extras onto preceding NoOps on the same engine (ge-imm waits commute)."""
    for f in nc.m.functions:
        for bb in f.blocks:
            newl = []
            changed = False
            for inst in list(bb.instructions):
                si = inst.sync_info
                waits = list(si.on_wait) if si and si.on_wait else []
                if len(waits) > 1 and inst.engine in _ENGINES:
                    changed = True
                    for k, w in enumerate(waits[:-1]):
                        newl.append(
                            mybir.InstNoOp(
                                name=f"{inst.name}-ws{k}",
                                ins=[],
                                outs=[],
                                engine=inst.engine,
                                sync_info=mybir.SyncInfo(on_wait=[w], on_update=[]),
                            )
                        )
                    si.on_wait = waits[-1:]
                    inst.sync_info = si
                newl.append(inst)
            if changed:
                bb.instructions = newl


def build_program(apply_bias=False, apply_gb=False):
    nc = bass.Bass(trn_type="TRN2")
    caps = nc.dram_tensor("caps", [16, 128, XW], BF16, kind="ExternalInput")
    w = nc.dram_tensor("w", [128, 768], BF16, kind="ExternalInput")
    maskd = nc.dram_tensor("maskd", [128, 8], BF16, kind="ExternalInput")
    identb = nc.dram_tensor("identb", [128, 128], BF16, kind="ExternalInput")
    identf = nc.dram_tensor("identf", [128, 128], F32, kind="ExternalInput")
    if apply_bias:
        cb = nc.dram_tensor("cb", [128, 1], F32, kind="ExternalInput")
    if apply_gb:
        gam = nc.dram_tensor("gam", [1, 16], F32, kind="ExternalInput")
        bet = nc.dram_tensor("bet", [1, 16], F32, kind="ExternalInput")
    out = nc.dram_tensor("out", [2, 128, 1024], F32, kind="ExternalOutput")

    def hview(ap_flat, o, rows):
        """[P, rows, 32] window at flat offset o with padded row stride 34."""
        return ap_flat[:, o: o + rows * 34].rearrange(
            "c (h w) -> c h w", w=34
        )[:, :, 0:32]

    with tile.TileContext(nc) as tc:
        with ExitStack() as ctx:
            singles = ctx.enter_context(tc.tile_pool(name="singles", bufs=1))
            ps_conv = ctx.enter_context(tc.tile_pool(name="ps_conv", bufs=2, space="PSUM"))
            ps_tr = ctx.enter_context(tc.tile_pool(name="ps_tr", bufs=2, space="PSUM"))
            ps_acc = ctx.enter_context(tc.tile_pool(name="ps_acc", bufs=1, space="PSUM"))
            usqp = ctx.enter_context(tc.tile_pool(name="usqp", bufs=1))
            big = ctx.enter_context(tc.tile_pool(name="big", bufs=2))
            sm = ctx.enter_context(tc.tile_pool(name="sm", bufs=2))
            vout = ctx.enter_context(tc.tile_pool(name="vout", bufs=2))

            w_sb = singles.tile([128, 768], BF16, tag="w")
            nc.sync.dma_start(out=w_sb[:], in_=w.ap())
            mk_sb = singles.tile([128, 8], BF16, tag="maskd")
            nc.sync.dma_start(out=mk_sb[:], in_=maskd.ap())
            idb_sb = singles.tile([128, 128], BF16, tag="identb")
            nc.sync.dma_start(out=idb_sb[:], in_=identb.ap())
            idf_sb = singles.tile([128, 128], F32, tag="identf")
            nc.sync.dma_start(out=idf_sb[:], in_=identf.ap())
            if apply_bias:
                cb_sb = singles.tile([128, 1], F32, tag="cb")
                nc.sync.dma_start(out=cb_sb[:], in_=cb.ap())
            if apply_gb:
                gam_sb = singles.tile([128, 16], F32, tag="gam")
                nc.sync.dma_start(out=gam_sb[:], in_=gam.ap().partition_broadcast(128))
                bet_sb = singles.tile([128, 16], F32, tag="bet")
                nc.sync.dma_start(out=bet_sb[:], in_=bet.ap().partition_broadcast(128))

            eps5 = singles.tile([128, 1], F32, tag="eps5")
            nc.vector.memset(eps5[:], 1e-5)
            eps16 = singles.tile([128, 1], F32, tag="eps16")
            nc.vector.memset(eps16[:], 1e-16)

            xp_sb = []
            for j in range(16):
                xj = singles.tile([128, XW], BF16, tag=f"x{j}", name=f"x{j}")
                nc.sync.dma_start(out=xj[:], in_=caps.ap()[j])
                xp_sb.append(xj)

            u_sb = [
                singles.tile([128, 1024], BF16, tag=f"u{j}", name=f"u{j}")
                for j in range(16)
            ]

            for lb in range(2):
                # ---- conv for the 8 images of this b (128 out ch each) ----
                for i in range(8):
                    j = lb * 8 + i
                    xp = xp_sb[j]
                    for chh in range(2):
                        ps = ps_conv.tile([128, 512], F32, tag="ps")
                        base = chh * 544
                        for kx in range(3):
                            nc.tensor.matmul(
                                ps[:],
                                lhsT=w_sb[:, kx * 128:(kx + 1) * 128],
                                rhs=hview(xp, base + kx, 16),
                                start=(kx == 0),
                                stop=False,
                            )
                        for kx in range(3):
                            nc.tensor.matmul(
                                ps[:],
                                lhsT=w_sb[0:64, 384 + kx * 128: 384 + (kx + 1) * 128],
                                rhs=hview(xp[0:64], base + 68 + kx, 16),
                                start=False,
                                stop=(kx == 2),
                            )
                        dst = u_sb[j][:, chh * 512:(chh + 1) * 512]
                        if apply_bias:
                            nc.scalar.activation(dst, ps[:], AF.Identity, bias=cb_sb[:], scale=1.0)
                        else:
                            nc.scalar.activation(dst, ps[:], AF.Copy)

                # u^2 (channel-major) for the msq statistics matmuls
                usq = []
                for i in range(8):
                    j = lb * 8 + i
                    uq = usqp.tile([128, 1024], BF16, tag=f"usq{i}", name=f"usq{i}")
                    eng = nc.vector if i < 6 else nc.gpsimd
                    eng.tensor_tensor(out=uq[:], in0=u_sb[j][:], in1=u_sb[j][:], op=OP.mult)
                    usq.append(uq)

                v_ch = vout.tile([128, 1024], F32, tag="v_ch")

                for sh in range(2):
                    # ---- PE: transposed pos-major blocks + LN stats ----
                    # pst[i]: [128 pos, (k, n', g, d)] bf16 per image
                    pst = []
                    for i in range(8):
                        j = lb * 8 + i
                        pt = ps_tr.tile([128, 512], BF16, tag="pst")
                        for k in range(4):
                            nc.tensor.transpose(
                                out=pt[:, k * 128:(k + 1) * 128],
                                in_=u_sb[j][:, (sh * 4 + k) * 128:(sh * 4 + k + 1) * 128],
                                identity=idb_sb[:],
                            )
                        pst.append(pt)

                    # mu/msq sums over d: [128 pos, (k, i, n', g)] via mask-matmuls
                    mus = ps_acc.tile([128, 512], F32, tag="mus")
                    for i in range(8):
                        j = lb * 8 + i
                        for k in range(4):
                            c0 = k * 64 + i * 8
                            nc.tensor.matmul(
                                mus[:, c0:c0 + 8],
                                lhsT=u_sb[j][:, (sh * 4 + k) * 128:(sh * 4 + k + 1) * 128],
                                rhs=mk_sb[:],
                                start=True, stop=True,
                            )
                            c1 = 256 + c0
                            nc.tensor.matmul(
                                mus[:, c1:c1 + 8],
                                lhsT=usq[i][:, (sh * 4 + k) * 128:(sh * 4 + k + 1) * 128],
                                rhs=mk_sb[:],
                                start=True, stop=True,
                            )

                    # ---- site stats smalls ([128, 256], cols (k,i,n',g)) ----
                    m1 = sm.tile([128, 256], F32, tag="m1")
                    nc.vector.tensor_scalar_mul(out=m1[:], in0=mus[:, 0:256], scalar1=1.0 / 16.0)
                    var = sm.tile([128, 256], F32, tag="var")
                    nc.vector.tensor_tensor(out=var[:], in0=m1[:], in1=m1[:], op=OP.mult)
                    nc.vector.scalar_tensor_tensor(
                        out=var[:], in0=mus[:, 256:512], scalar=1.0 / 16.0, in1=var[:],
                        op0=OP.mult, op1=OP.subtract,
                    )
                    rstd = sm.tile([128, 256], F32, tag="rstd")
                    nc.scalar.activation(rstd[:], var[:], AF.Sqrt, bias=eps5[:])
                    nc.vector.reciprocal(rstd[:], rstd[:])
                    rstd_b = sm.tile([128, 256], BF16, tag="rstd_b")
                    nc.gpsimd.tensor_scalar_mul(out=rstd_b[:], in0=rstd[:], scalar1=1.0)
                    n2 = sm.tile([128, 256], F32, tag="n2")
                    nc.vector.tensor_tensor(out=n2[:], in0=m1[:], in1=rstd[:], op=OP.mult)

                    # ---- P = T * rstd  (pos-major bf16, cols (k, i, n'gd)) ----
                    P = big.tile([128, 4096], BF16, tag="P")
                    P4 = P.rearrange("p (k i c) -> p k i c", k=4, i=8)
                    # rstd_b cols (k, i, n'g): per image -> [p, k, (n'g), d-bc]
                    rb4 = rstd_b.rearrange("p (k i e) -> p k i e", k=4, i=8)
                    for i in range(8):
                        nc.vector.tensor_tensor(
                            out=P4[:, :, i].rearrange("p k (e d) -> p k e d", d=16),
                            in0=pst[i].rearrange("p (k e d) -> p k e d", k=4, d=16),
                            in1=rb4[:, :, i].unsqueeze(3).broadcast_to((128, 4, 8, 16)),
                            op=OP.mult,
                        )

                    if apply_gb:
                        # up' = (P - n2)*gamma + beta, then use uncorrected formulas
                        n2b = sm.tile([128, 256], BF16, tag="n2b")
                        nc.vector.tensor_scalar_mul(out=n2b[:], in0=n2[:], scalar1=1.0)
                        UP = big.tile([128, 4096], BF16, tag="UP")
                        UP3 = UP.rearrange("p (m d) -> p m d", d=16)
                        # n2 cols (k,i,n',g) fully merge to m=256
                        nc.vector.tensor_tensor(
                            out=UP3, in0=P.rearrange("p (m d) -> p m d", d=16),
                            in1=n2b[:].unsqueeze(2).broadcast_to((128, 256, 16)),
                            op=OP.subtract,
                        )
                        g_bc = gam_sb[:].unsqueeze(1).broadcast_to((128, 256, 16))
                        b_bc = bet_sb[:].unsqueeze(1).broadcast_to((128, 256, 16))
                        nc.vector.tensor_tensor(out=UP3, in0=UP3, in1=g_bc, op=OP.mult)
                        nc.vector.tensor_tensor(out=UP3, in0=UP3, in1=b_bc, op=OP.add)
                        P = UP
                        P4 = P.rearrange("p (k i c) -> p k i c", k=4, i=8)

                    # ---- S = sum_i P (PE accumulate), minus c = sum_i n2 ----
                    S_ps = ps_acc.tile([128, 512], F32, tag="S_ps")
                    S_ps3 = S_ps.rearrange("p (k c) -> p k c", k=4)
                    for i in range(8):
                        nc.tensor.matmul(
                            S_ps3,
                            lhsT=idb_sb[:],
                            rhs=P4[:, :, i],
                            start=(i == 0), stop=(i == 7),
                        )
                    S_sb = sm.tile([128, 512], BF16, tag="S_sb")
                    # views with cols (k, n'g, d)
                    S4 = S_sb.rearrange("p (k e d) -> p k e d", k=4, d=16)
                    if apply_gb:
                        nc.vector.tensor_scalar_mul(out=S_sb[:], in0=S_ps[:], scalar1=1.0)
                    else:
                        c_s = sm.tile([128, 32], F32, tag="c_s")
                        c_s3 = c_s.rearrange("p (k e) -> p k e", k=4)
                        n2_4 = n2.rearrange("p (k i e) -> p k i e", k=4, i=8)
                        nc.vector.reduce_sum(c_s3, n2_4.transpose((0, 1, 3, 2)), AX.X)
                        nc.vector.tensor_tensor(
                            out=S4,
                            in0=S_ps.rearrange("p (k e d) -> p k e d", k=4, d=16),
                            in1=c_s3.unsqueeze(3).broadcast_to((128, 4, 8, 16)),
                            op=OP.subtract,
                        )

                    # ---- dot_i = sum_d P*S (tree over innermost d) ----
                    Q = big.tile([128, 4096], BF16, tag="Q")
                    Q4 = Q.rearrange("p (k i c) -> p k i c", k=4, i=8)
                    nc.vector.tensor_tensor(
                        out=Q4, in0=P4,
                        in1=S_sb.rearrange("p (k c) -> p k c", k=4)
                            .unsqueeze(2).broadcast_to((128, 4, 8, 128)),
                        op=OP.mult,
                    )
                    Qt = Q.rearrange("p (m e d) -> p m e d", m=32, d=16)
                    for dd in (8, 4, 2, 1):
                        nc.vector.tensor_tensor(
                            out=Qt[:, :, :, 0:dd],
                            in0=Qt[:, :, :, 0:dd],
                            in1=Qt[:, :, :, dd:2 * dd],
                            op=OP.add,
                        )
                    dotp = sm.tile([128, 256], F32, tag="dotp")
                    nc.gpsimd.tensor_scalar_mul(
                        out=dotp.rearrange("p (m e) -> p m e", m=32),
                        in0=Qt[:, :, :, 0], scalar1=1.0)

                    # rr = (dot - n2*sum_d S) / norm_sq ; norm_sq = 16*var*rstd^2
                    rr = sm.tile([128, 256], F32, tag="rr")
                    if apply_gb:
                        R2 = big.tile([128, 4096], BF16, tag="Q", name="R2")
                        R2t = R2.rearrange("p (m e d) -> p m e d", m=32, d=16)
                        nc.vector.tensor_tensor(
                            out=R2.rearrange("p (m d) -> p m d", d=16),
                            in0=P.rearrange("p (m d) -> p m d", d=16),
                            in1=P.rearrange("p (m d) -> p m d", d=16), op=OP.mult)
                        for dd in (8, 4, 2, 1):
                            nc.vector.tensor_tensor(
                                out=R2t[:, :, :, 0:dd],
                                in0=R2t[:, :, :, 0:dd],
                                in1=R2t[:, :, :, dd:2 * dd],
                                op=OP.add,
                            )
                        nsq_i = sm.tile([128, 256], F32, tag="nsq_i")
                        nc.vector.tensor_scalar_mul(
                            out=nsq_i.rearrange("p (m e) -> p m e", m=32),
                            in0=R2t[:, :, :, 0], scalar1=1.0)
                        nc.vector.tensor_scalar_max(out=nsq_i[:], in0=nsq_i[:], scalar1=1e-8)
                        nc.vector.reciprocal(nsq_i[:], nsq_i[:])
                        nc.vector.tensor_tensor(out=rr[:], in0=dotp[:], in1=nsq_i[:], op=OP.mult)
                    else:
                        ssum = sm.tile([128, 32], F32, tag="ssum")
                        ssum3 = ssum.rearrange("p (k e) -> p k e", k=4)
                        nc.vector.reduce_sum(ssum3, S4, AX.X)
                        t1 = sm.tile([128, 256], F32, tag="t1")
                        nc.vector.tensor_tensor(
                            out=t1.rearrange("p (k i e) -> p k i e", k=4, i=8),
                            in0=n2.rearrange("p (k i e) -> p k i e", k=4, i=8),
                            in1=ssum3.unsqueeze(2).broadcast_to((128, 4, 8, 8)),
                            op=OP.mult,
                        )
                        nc.vector.tensor_tensor(out=dotp[:], in0=dotp[:], in1=t1[:], op=OP.subtract)
                        ns = sm.tile([128, 256], F32, tag="ns")
                        nc.vector.tensor_tensor(out=ns[:], in0=rstd[:], in1=rstd[:], op=OP.mult)
                        nc.vector.scalar_tensor_tensor(
                            out=ns[:], in0=var[:], scalar=16.0, in1=ns[:],
                            op0=OP.mult, op1=OP.mult,
                        )
                        nc.vector.reciprocal(ns[:], ns[:])
                        nc.vector.tensor_tensor(out=rr[:], in0=dotp[:], in1=ns[:], op=OP.mult)

                    # ---- softmax over i (cols (k, i, n'g)) ----
                    rr4 = rr.rearrange("p (k i e) -> p k i e", k=4, i=8)
                    mx = sm.tile([128, 32], F32, tag="mx")
                    mx3 = mx.rearrange("p (k e) -> p k e", k=4)
                    nc.vector.reduce_max(mx3, rr4.transpose((0, 1, 3, 2)), AX.X)
                    es = sm.tile([128, 256], F32, tag="es")
                    es4 = es.rearrange("p (k i e) -> p k i e", k=4, i=8)
                    nc.vector.tensor_tensor(
                        out=es4, in0=rr4,
                        in1=mx3.unsqueeze(2).broadcast_to((128, 4, 8, 8)),
                        op=OP.subtract,
                    )
                    nc.scalar.activation(es[:], es[:], AF.Exp)
                    Z = sm.tile([128, 32], F32, tag="Z")
                    Z3 = Z.rearrange("p (k e) -> p k e", k=4)
                    nc.vector.reduce_sum(Z3, es4.transpose((0, 1, 3, 2)), AX.X)
                    nc.vector.reciprocal(Z[:], Z[:])
                    sc_b = sm.tile([128, 256], BF16, tag="sc_b")
                    nc.vector.tensor_tensor(
                        out=sc_b.rearrange("p (k i e) -> p k i e", k=4, i=8),
                        in0=es4,
                        in1=Z3.unsqueeze(2).broadcast_to((128, 4, 8, 8)),
                        op=OP.mult,
                    )

                    # ---- s = sum_i sc_i * P (PE accumulate) with correction ----
                    R = big.tile([128, 4096], BF16, tag="Q", name="R")
                    nc.vector.tensor_tensor(
                        out=R.rearrange("p (m d) -> p m d", d=16),
                        in0=P.rearrange("p (m d) -> p m d", d=16),
                        in1=sc_b[:].unsqueeze(2).broadcast_to((128, 256, 16)),
                        op=OP.mult,
                    )
                    R4 = R.rearrange("p (k i c) -> p k i c", k=4, i=8)
                    s_ps = ps_acc.tile([128, 512], F32, tag="s_ps")
                    s_ps3 = s_ps.rearrange("p (k c) -> p k c", k=4)
                    for i in range(8):
                        nc.tensor.matmul(
                            s_ps3,
                            lhsT=idb_sb[:],
                            rhs=R4[:, :, i],
                            start=(i == 0), stop=(i == 7),
                        )
                    s_sb = sm.tile([128, 512], F32, tag="s_sb")
                    s4 = s_sb.rearrange("p (k e d) -> p k e d", k=4, d=16)
                    if apply_gb:
                        nc.vector.tensor_scalar_mul(out=s_sb[:], in0=s_ps[:], scalar1=1.0)
                    else:
                        t2 = sm.tile([128, 256], F32, tag="t2")
                        nc.vector.tensor_tensor(out=t2[:], in0=sc_b[:], in1=n2[:], op=OP.mult)
                        csc = sm.tile([128, 32], F32, tag="csc")
                        csc3 = csc.rearrange("p (k e) -> p k e", k=4)
                        t2_4 = t2.rearrange("p (k i e) -> p k i e", k=4, i=8)
                        nc.vector.reduce_sum(csc3, t2_4.transpose((0, 1, 3, 2)), AX.X)
                        nc.vector.tensor_tensor(
                            out=s4,
                            in0=s_ps.rearrange("p (k e d) -> p k e d", k=4, d=16),
                            in1=csc3.unsqueeze(3).broadcast_to((128, 4, 8, 16)),
                            op=OP.subtract,
                        )

                    # ---- squash over g (cols (k, n', g, d)) ----
                    ssq = sm.tile([128, 512], F32, tag="ssq")
                    nc.scalar.activation(ssq[:], s_sb[:], AF.Square)
                    nsq = sm.tile([128, 128], F32, tag="nsq")
                    nsq4 = nsq.rearrange("p (k n d) -> p k n d", k=4, n=2)
                    ssq5 = ssq.rearrange("p (k n g d) -> p k n g d", k=4, n=2, g=4)
                    nc.vector.reduce_sum(nsq4, ssq5.transpose((0, 1, 2, 4, 3)), AX.X)
                    sq1 = sm.tile([128, 128], F32, tag="sq1")
                    nc.scalar.activation(sq1[:], nsq[:], AF.Sqrt, bias=eps16[:])
                    nc.vector.scalar_tensor_tensor(
                        out=sq1[:], in0=nsq[:], scalar=1.0, in1=sq1[:],
                        op0=OP.add, op1=OP.mult,
                    )
                    nc.vector.reciprocal(sq1[:], sq1[:])
                    f = sm.tile([128, 128], F32, tag="f")
                    nc.vector.tensor_tensor(out=f[:], in0=nsq[:], in1=sq1[:], op=OP.mult)

                    v_sb = sm.tile([128, 512], F32, tag="v_sb")
                    f4 = f.rearrange("p (k n d) -> p k n d", k=4, n=2)
                    nc.vector.tensor_tensor(
                        out=v_sb.rearrange("p (k n g d) -> p k n g d", k=4, n=2, g=4),
                        in0=s_sb.rearrange("p (k n g d) -> p k n g d", k=4, n=2, g=4),
                        in1=f4.unsqueeze(3).broadcast_to((128, 4, 2, 4, 16)),
                        op=OP.mult,
                    )

                    # ---- back to channel-major and stage into v_ch ----
                    vt = ps_acc.tile([128, 512], F32, tag="vt")
                    for k in range(4):
                        nc.tensor.transpose(
                            out=vt[:, k * 128:(k + 1) * 128],
                            in_=v_sb[:, k * 128:(k + 1) * 128],
                            identity=idf_sb[:],
                        )
                    nc.scalar.activation(v_ch[:, sh * 512:(sh + 1) * 512], vt[:], AF.Copy)

                nc.sync.dma_start(out=out.ap()[lb], in_=v_ch[:])

    _split_sync_waits(nc)
    return nc


def _prep_caps(capsules):
    """caps2[h]: [16, 128, XW] bf16; j = lb*8 + i -> image (b=2h+lb, nin=i).
    p 0-63: padded image; p 64-127: same shifted up one padded row."""
    import ml_dtypes
    caps = np.asarray(capsules, np.float32)  # [4, 8, 16, 4, 32, 32]
    B = caps.reshape(4, 8, 64, 32, 32)
    halves = []
    for h in range(2):
        arr = np.zeros((16, 128, XW), np.float32)
        for lb in range(2):
            for i in range(8):
                img = B[2 * h + lb, i]  # [64, 32, 32]
                pad = np.zeros((64, 34, 34), np.float32)
                pad[:, 1:33, 1:33] = img
                flat = pad.reshape(64, 1156)
                j = lb * 8 + i
                arr[j, 0:64, 0:1156] = flat
                arr[j, 64:128, 0:1122] = flat[:, 34:]
        halves.append(arr.astype(ml_dtypes.bfloat16))
    return halves


_CACHE = {}


def kernel(capsules, conv_w, conv_b, ln_gamma, ln_beta):
    import ml_dtypes
    conv_b = np.asarray(conv_b, np.float32)
    ln_gamma = np.asarray(ln_gamma, np.float32)
    ln_beta = np.asarray(ln_beta, np.float32)
    apply_bias = bool(np.any(conv_b))
    apply_gb = bool(np.any(ln_gamma != 1.0) or np.any(ln_beta != 0.0))

    key = (apply_bias, apply_gb)
    if key not in _CACHE:
        _CACHE[key] = build_program(apply_bias=apply_bias, apply_gb=apply_gb)
    nc = _CACHE[key]

    w = np.asarray(conv_w, np.float32)
    wt = np.stack(
        [np.roll(np.rot90(w, k=r, axes=(3, 4)), r, axis=2) for r in range(4)], axis=1
    )
    W512 = np.ascontiguousarray(wt.reshape(512, 64, 3, 3), dtype=np.float32)
    # output-channel order within the pair: m = n'*64 + d*4 + g
    m_idx = np.arange(128)
    m_n = m_idx // 64
    m_d = (m_idx // 4) % 16
    m_g = m_idx % 4
    wpacks = []
    for p in range(4):
        Wc = W512[128 * p: 128 * p + 128]  # [128 outch, 64 inch, 3, 3]
        wp = np.zeros((128, 768), np.float32)
        for kx in range(3):
            wp[0:64, kx * 128:(kx + 1) * 128] = Wc[:, :, 0, kx].T
            wp[64:128, kx * 128:(kx + 1) * 128] = Wc[:, :, 1, kx].T
            wp[0:64, 384 + kx * 128: 384 + (kx + 1) * 128] = Wc[:, :, 2, kx].T
        wpacks.append(np.ascontiguousarray(wp.astype(ml_dtypes.bfloat16)))

    caps_halves = _prep_caps(capsules)

    maskd = np.zeros((128, 8), np.float32)
    maskd[m_idx, m_n * 4 + m_g] = 1.0
    maskd = np.ascontiguousarray(maskd.astype(ml_dtypes.bfloat16))
    identb = np.ascontiguousarray(np.eye(128).astype(ml_dtypes.bfloat16))
    identf = np.eye(128, dtype=np.float32)

    in_maps = []
    for c in range(8):
        p, h = c // 2, c % 2
        m = {
            "caps": caps_halves[h],
            "w": wpacks[p],
            "maskd": maskd,
            "identb": identb,
            "identf": identf,
        }
        if apply_bias:
            cb = conv_b[(2 * p + m_n) * 16 + m_d].reshape(128, 1).astype(np.float32)
            m["cb"] = np.ascontiguousarray(cb)
        if apply_gb:
            m["gam"] = np.ascontiguousarray(ln_gamma.reshape(1, 16))
            m["bet"] = np.ascontiguousarray(ln_beta.reshape(1, 16))
        in_maps.append(m)

    res = run_bass_kernel_spmd(nc, in_maps, core_ids=list(range(8)), trace=False)
    full = np.zeros((4, 8, 16, 4, 32, 32), np.float32)
    for c in range(8):
        p, h = c // 2, c % 2
        o = np.asarray(res.results[c]["out"], np.float32).reshape(2, 2, 16, 4, 32, 32)
        full[2 * h: 2 * h + 2, 2 * p: 2 * p + 2] = o
    return full


# revision 15
# speedup vs baseline: 2.1969x; 1.0560x over previous
"""Trainium2 Bass kernel for nn_ConvolutionalCapsules.

Sharding: 8 cores = 4 nout-pairs x 2 batch-halves. Core (p, h) runs the p4
group conv for output channels of nouts {2p, 2p+1} (M=128) over the 16 images
(b in {2h, 2h+1}) x (nin 0..7), then LayerNorm + degree-score routing + squash
for those (2 b, 2 nout) blocks.

Conv: 3x3 conv as shifted matmuls from a host-prepadded doubled image in bf16:
partitions 0-63 hold the zero-padded 34x34 image, partitions 64-127 the same
image shifted up one padded row, so one K=128 matmul covers taps (0,kx)+(1,kx);
row-2 taps run as K=64 matmuls on the first half. 6 matmuls accumulate one
[128, 512] PSUM tile (128 out channels x 512 positions).

Output channels are ordered (n', d, g); broadcast operands keep a real
stride-1 innermost dim (g) so DVE bf16 TTs stay in the 2x perf mode, with
merged views keeping every AP at <=4 free dims.

Routing: LN statistics (sum over d) are computed by PE mask-matmuls straight
from the channel-major conv output (lhsT = u block, rhs = 8-col group mask,
~free on PE). Elementwise work runs pos-major in bf16 on DVE (2x mode) on
PE-transposed blocks; sums over the 8 input capsules use PE identity-matmul
PSUM accumulation. The LN centering is folded algebraically:
  P = T*rstd,  up = P - n2   (n2 = mu*rstd per site)
  S = sum_i P - (sum_i n2);  dot_i = sum_d P*S - n2_i * sum_d S
  s = sum_i sc_i*P - sum_i sc_i*n2_i
so no explicit centered tensor is materialized.
"""

import numpy as np
from contextlib import ExitStack

import concourse.bass as bass
import concourse.tile as tile
from concourse import mybir
from concourse.bass_utils import run_bass_kernel_spmd

F32 = mybir.dt.float32
BF16 = mybir.dt.bfloat16
F32R = mybir.dt.float32r
AF = mybir.ActivationFunctionType
OP = mybir.AluOpType
AX = mybir.AxisListType

XW = 1164  # padded 34x34 image (1156) + slack so 16-row AP views stay in-bounds

_ENGINES = {
    mybir.EngineType.PE,
    mybir.EngineType.Activation,
    mybir.EngineType.Pool,
    mybir.EngineType.DVE,
    mybir.EngineType.SP,
}


def _split_sync_waits(nc):
    """This walrus build accepts a single embedded sync-wait per instruction;
    hoist extras onto preceding NoOps on the same engine (ge-imm waits commute)."""
    for f in nc.m.functions:
        for bb in f.blocks:
            newl = []
            changed = False
            for inst in list(bb.instructions):
                si = inst.sync_info
                waits = list(si.on_wait) if si and si.on_wait else []
                if len(waits) > 1 and inst.engine in _ENGINES:
                    changed = True
                    for k, w in enumerate(waits[:-1]):
                        newl.append(
                            mybir.InstNoOp(
                                name=f"{inst.name}-ws{k}",
                                ins=[],
                                outs=[],
                                engine=inst.engine,
                                sync_info=mybir.SyncInfo(on_wait=[w], on_update=[]),
                            )
                        )
                    si.on_wait = waits[-1:]
                    inst.sync_info = si
                newl.append(inst)
            if changed:
                bb.instructions = newl


def build_program(apply_bias=False, apply_gb=False):
    nc = bass.Bass(trn_type="TRN2")
    caps = nc.dram_tensor("caps", [16, 128, XW], F32R, kind="ExternalInput")
    w = nc.dram_tensor("w", [128, 768], F32R, kind="ExternalInput")
    maskd = nc.dram_tensor("maskd", [128, 8], BF16, kind="ExternalInput")
    identb = nc.dram_tensor("identb", [128, 128], BF16, kind="ExternalInput")
    identf = nc.dram_tensor("identf", [128, 128], F32, kind="ExternalInput")
    if apply_bias:
        cb = nc.dram_tensor("cb", [128, 1], F32, kind="ExternalInput")
    if apply_gb:
        gam = nc.dram_tensor("gam", [1, 16], F32, kind="ExternalInput")
        bet = nc.dram_tensor("bet", [1, 16], F32, kind="ExternalInput")
    out = nc.dram_tensor("out", [2, 128, 1024], F32, kind="ExternalOutput")

    def hview(ap_flat, o, rows):
        """[P, rows, 32] window at flat offset o with padded row stride 34."""
        return ap_flat[:, o: o + rows * 34].rearrange(
            "c (h w) -> c h w", w=34
        )[:, :, 0:32]

    with tile.TileContext(nc) as tc:
        with ExitStack() as ctx:
            singles = ctx.enter_context(tc.tile_pool(name="singles", bufs=1))
            ps_conv = ctx.enter_context(tc.tile_pool(name="ps_conv", bufs=2, space="PSUM"))
            ps_tr = ctx.enter_context(tc.tile_pool(name="ps_tr", bufs=2, space="PSUM"))
            ps_acc = ctx.enter_context(tc.tile_pool(name="ps_acc", bufs=1, space="PSUM"))
            usqp = ctx.enter_context(tc.tile_pool(name="usqp", bufs=1))
            big = ctx.enter_context(tc.tile_pool(name="big", bufs=2))
            sm = ctx.enter_context(tc.tile_pool(name="sm", bufs=2))
            vout = ctx.enter_context(tc.tile_pool(name="vout", bufs=2))
            xpool = ctx.enter_context(tc.tile_pool(name="xpool", bufs=4))
            trp = ctx.enter_context(tc.tile_pool(name="trp", bufs=1))

            w_sb = singles.tile([128, 768], F32R, tag="w")
            nc.sync.dma_start(out=w_sb[:], in_=w.ap())
            mk_sb = singles.tile([128, 8], BF16, tag="maskd")
            nc.sync.dma_start(out=mk_sb[:], in_=maskd.ap())
            idb_sb = singles.tile([128, 128], BF16, tag="identb")
            nc.sync.dma_start(out=idb_sb[:], in_=identb.ap())
            idf_sb = singles.tile([128, 128], F32, tag="identf")
            nc.sync.dma_start(out=idf_sb[:], in_=identf.ap())
            if apply_bias:
                cb_sb = singles.tile([128, 1], F32, tag="cb")
                nc.sync.dma_start(out=cb_sb[:], in_=cb.ap())
            if apply_gb:
                gam_sb = singles.tile([128, 16], F32, tag="gam")
                nc.sync.dma_start(out=gam_sb[:], in_=gam.ap().partition_broadcast(128))
                bet_sb = singles.tile([128, 16], F32, tag="bet")
                nc.sync.dma_start(out=bet_sb[:], in_=bet.ap().partition_broadcast(128))

            eps5 = singles.tile([128, 1], F32, tag="eps5")
            nc.vector.memset(eps5[:], 1e-5)
            eps16 = singles.tile([128, 1], F32, tag="eps16")
            nc.vector.memset(eps16[:], 1e-16)

            u_sb = [
                singles.tile([128, 1024], BF16, tag=f"u{j}", name=f"u{j}")
                for j in range(16)
            ]

            for lb in range(2):
                # ---- conv for the 8 images of this b (128 out ch each) ----
                for i in range(8):
                    j = lb * 8 + i
                    xp = xpool.tile([128, XW], F32R, tag="xp", name=f"x{j}")
                    nc.sync.dma_start(out=xp[:], in_=caps.ap()[j])
                    for chh in range(2):
                        ps = ps_conv.tile([128, 512], F32, tag="ps")
                        base = chh * 544
                        for kx in range(3):
                            nc.tensor.matmul(
                                ps[:],
                                lhsT=w_sb[:, kx * 128:(kx + 1) * 128],
                                rhs=hview(xp, base + kx, 16),
                                start=(kx == 0),
                                stop=False,
                            )
                        for kx in range(3):
                            nc.tensor.matmul(
                                ps[:],
                                lhsT=w_sb[0:64, 384 + kx * 128: 384 + (kx + 1) * 128],
                                rhs=hview(xp[0:64], base + 68 + kx, 16),
                                start=False,
                                stop=(kx == 2),
                            )
                        dst = u_sb[j][:, chh * 512:(chh + 1) * 512]
                        if apply_bias:
                            nc.scalar.activation(dst, ps[:], AF.Identity, bias=cb_sb[:], scale=1.0)
                        else:
                            nc.scalar.activation(dst, ps[:], AF.Copy)

                # u^2 (channel-major) for the msq statistics matmuls
                usq = []
                for i in range(8):
                    j = lb * 8 + i
                    uq = usqp.tile([128, 1024], BF16, tag=f"usq{i}", name=f"usq{i}")
                    eng = nc.vector if i < 6 else nc.gpsimd
                    eng.tensor_tensor(out=uq[:], in0=u_sb[j][:], in1=u_sb[j][:], op=OP.mult)
                    usq.append(uq)

                v_ch = vout.tile([128, 1024], F32, tag="v_ch")

                for sh in range(2):
                    # ---- PE: transposed pos-major blocks + LN stats ----
                    # pst[i]: [128 pos, (k, n', d, g)] bf16 per image
                    pst = []
                    for i in range(8):
                        j = lb * 8 + i
                        pt = ps_tr.tile([128, 512], BF16, tag="pst")
                        for k in range(4):
                            nc.tensor.transpose(
                                out=pt[:, k * 128:(k + 1) * 128],
                                in_=u_sb[j][:, (sh * 4 + k) * 128:(sh * 4 + k + 1) * 128],
                                identity=idb_sb[:],
                            )
                        pst.append(pt)

                    # mu/msq sums over d: [128 pos, (k, i, n', g)] via mask-matmuls
                    mus = ps_acc.tile([128, 512], F32, tag="mus")
                    for i in range(8):
                        j = lb * 8 + i
                        for k in range(4):
                            c0 = k * 64 + i * 8
                            nc.tensor.matmul(
                                mus[:, c0:c0 + 8],
                                lhsT=u_sb[j][:, (sh * 4 + k) * 128:(sh * 4 + k + 1) * 128],
                                rhs=mk_sb[:],
                                start=True, stop=True,
                            )
                            c1 = 256 + c0
                            nc.tensor.matmul(
                                mus[:, c1:c1 + 8],
                                lhsT=usq[i][:, (sh * 4 + k) * 128:(sh * 4 + k + 1) * 128],
                                rhs=mk_sb[:],
                                start=True, stop=True,
                            )

                    # ---- site stats smalls ([128, 256], cols (k,i,n,g)) ----
                    m1 = sm.tile([128, 256], F32, tag="m1")
                    nc.vector.tensor_scalar_mul(out=m1[:], in0=mus[:, 0:256], scalar1=1.0 / 16.0)
                    var = sm.tile([128, 256], F32, tag="var")
                    nc.vector.tensor_tensor(out=var[:], in0=m1[:], in1=m1[:], op=OP.mult)
                    nc.vector.scalar_tensor_tensor(
                        out=var[:], in0=mus[:, 256:512], scalar=1.0 / 16.0, in1=var[:],
                        op0=OP.mult, op1=OP.subtract,
                    )
                    rstd = sm.tile([128, 256], F32, tag="rstd")
                    nc.scalar.activation(rstd[:], var[:], AF.Sqrt, bias=eps5[:])
                    nc.vector.reciprocal(rstd[:], rstd[:])
                    rstd_b = sm.tile([128, 256], BF16, tag="rstd_b")
                    nc.gpsimd.tensor_scalar_mul(out=rstd_b[:], in0=rstd[:], scalar1=1.0)
                    n2 = sm.tile([128, 256], F32, tag="n2")
                    nc.vector.tensor_tensor(out=n2[:], in0=m1[:], in1=rstd[:], op=OP.mult)

                    # ---- P = T * rstd  (pos-major bf16, cols (k, i, n', d, g)) ----
                    P = big.tile([128, 4096], BF16, tag="P")
                    P4 = P.rearrange("p (k i c) -> p k i c", k=4, i=8)
                    rb5 = rstd_b.rearrange("p (k i n g) -> p k i n g", k=4, i=8, n=2)
                    for i in range(8):
                        for n in range(2):
                            nc.vector.tensor_tensor(
                                out=P4[:, :, i].rearrange("p k (n d g) -> p k n d g", n=2, d=16)[:, :, n],
                                in0=pst[i].rearrange("p (k n d g) -> p k n d g", k=4, n=2, d=16)[:, :, n],
                                in1=rb5[:, :, i, n].unsqueeze(2).broadcast_to((128, 4, 16, 4)),
                                op=OP.mult,
                            )

                    if apply_gb:
                        # up' = (P - n2)*gamma + beta, then use uncorrected formulas
                        n2b = sm.tile([128, 256], BF16, tag="n2b")
                        nc.vector.tensor_scalar_mul(out=n2b[:], in0=n2[:], scalar1=1.0)
                        UP = big.tile([128, 4096], BF16, tag="UP")
                        UP3 = UP.rearrange("p (m d g) -> p m d g", m=64, d=16)
                        nc.vector.tensor_tensor(
                            out=UP3, in0=P.rearrange("p (m d g) -> p m d g", m=64, d=16),
                            in1=n2b.rearrange("p (m g) -> p m g", m=64)
                                .unsqueeze(2).broadcast_to((128, 64, 16, 4)),
                            op=OP.subtract,
                        )
                        g_bc = (gam_sb[:].unsqueeze(1).unsqueeze(3)
                                .broadcast_to((128, 64, 16, 4)))
                        b_bc = (bet_sb[:].unsqueeze(1).unsqueeze(3)
                                .broadcast_to((128, 64, 16, 4)))
                        nc.vector.tensor_tensor(out=UP3, in0=UP3, in1=g_bc, op=OP.mult)
                        nc.vector.tensor_tensor(out=UP3, in0=UP3, in1=b_bc, op=OP.add)
                        P = UP
                        P4 = P.rearrange("p (k i c) -> p k i c", k=4, i=8)

                    # ---- S = sum_i P (PE accumulate), minus c = sum_i n2 ----
                    S_ps = ps_acc.tile([128, 512], F32, tag="S_ps")
                    S_ps3 = S_ps.rearrange("p (k c) -> p k c", k=4)
                    for i in range(8):
                        nc.tensor.matmul(
                            S_ps3,
                            lhsT=idb_sb[:],
                            rhs=P4[:, :, i],
                            start=(i == 0), stop=(i == 7),
                        )
                    S_sb = sm.tile([128, 512], BF16, tag="S_sb")
                    S5 = S_sb.rearrange("p (k n d g) -> p k n d g", k=4, n=2, d=16)
                    if apply_gb:
                        nc.vector.tensor_scalar_mul(out=S_sb[:], in0=S_ps[:], scalar1=1.0)
                    else:
                        c_s = sm.tile([128, 32], F32, tag="c_s")
                        c_s3 = c_s.rearrange("p (k e) -> p k e", k=4)
                        n2_4 = n2.rearrange("p (k i e) -> p k i e", k=4, i=8)
                        nc.vector.reduce_sum(c_s3, n2_4.transpose((0, 1, 3, 2)), AX.X)
                        nc.vector.tensor_tensor(
                            out=S5,
                            in0=S_ps.rearrange("p (k n d g) -> p k n d g", k=4, n=2, d=16),
                            in1=c_s.rearrange("p (k n g) -> p k n g", k=4, n=2)
                                .unsqueeze(3).broadcast_to((128, 4, 2, 16, 4)),
                            op=OP.subtract,
                        )

                    # ---- dot_i = sum_d P*S: bf16 product, f32 tree over d ----
                    Q = big.tile([128, 4096], BF16, tag="Q")
                    Q4 = Q.rearrange("p (k i c) -> p k i c", k=4, i=8)
                    nc.vector.tensor_tensor(
                        out=Q4, in0=P4,
                        in1=S_sb.rearrange("p (k c) -> p k c", k=4)
                            .unsqueeze(2).broadcast_to((128, 4, 8, 128)),
                        op=OP.mult,
                    )
                    Qt = Q.rearrange("p (m n d g) -> p m n d g", m=32, n=2, d=16)
                    TR = trp.tile([128, 2048], F32, tag="TR")
                    TRt = TR.rearrange("p (m n d g) -> p m n d g", m=32, n=2, d=8)
                    nc.vector.tensor_tensor(
                        out=TRt, in0=Qt[:, :, :, 0:8], in1=Qt[:, :, :, 8:16], op=OP.add)
                    for dd in (4, 2, 1):
                        eng = nc.gpsimd if dd == 4 else nc.vector
                        eng.tensor_tensor(
                            out=TRt[:, :, :, 0:dd],
                            in0=TRt[:, :, :, 0:dd],
                            in1=TRt[:, :, :, dd:2 * dd],
                            op=OP.add,
                        )
                    dotp = sm.tile([128, 256], F32, tag="dotp")
                    nc.gpsimd.tensor_scalar_mul(
                        out=dotp.rearrange("p (m n g) -> p m n g", m=32, n=2),
                        in0=TRt[:, :, :, 0], scalar1=1.0)

                    # rr = (dot - n2*sum_d S) / norm_sq ; norm_sq = 16*var*rstd^2
                    rr = sm.tile([128, 256], F32, tag="rr")
                    if apply_gb:
                        R2 = big.tile([128, 4096], BF16, tag="Q", name="R2")
                        R2m = R2.rearrange("p (m d g) -> p m d g", m=64, d=16)
                        nc.vector.tensor_tensor(
                            out=R2m, in0=P.rearrange("p (m d g) -> p m d g", m=64, d=16),
                            in1=P.rearrange("p (m d g) -> p m d g", m=64, d=16), op=OP.mult)
                        R2t = R2.rearrange("p (m n d g) -> p m n d g", m=32, n=2, d=16)
                        TR2t = TRt
                        nc.vector.tensor_tensor(
                            out=TR2t, in0=R2t[:, :, :, 0:8], in1=R2t[:, :, :, 8:16], op=OP.add)
                        for dd in (4, 2, 1):
                            nc.vector.tensor_tensor(
                                out=TR2t[:, :, :, 0:dd],
                                in0=TR2t[:, :, :, 0:dd],
                                in1=TR2t[:, :, :, dd:2 * dd],
                                op=OP.add,
                            )
                        nsq_i = sm.tile([128, 256], F32, tag="nsq_i")
                        nc.vector.tensor_scalar_mul(
                            out=nsq_i.rearrange("p (m n g) -> p m n g", m=32, n=2),
                            in0=TR2t[:, :, :, 0], scalar1=1.0)
                        nc.vector.tensor_scalar_max(out=nsq_i[:], in0=nsq_i[:], scalar1=1e-8)
                        nc.vector.reciprocal(nsq_i[:], nsq_i[:])
                        nc.vector.tensor_tensor(out=rr[:], in0=dotp[:], in1=nsq_i[:], op=OP.mult)
                    else:
                        ssum = sm.tile([128, 32], F32, tag="ssum")
                        ssum3 = ssum.rearrange("p (k e) -> p k e", k=4)
                        ssum4 = ssum.rearrange("p (k n g) -> p k n g", k=4, n=2)
                        nc.vector.reduce_sum(ssum4, S5.transpose((0, 1, 2, 4, 3)), AX.X)
                        t1 = sm.tile([128, 256], F32, tag="t1")
                        nc.gpsimd.tensor_tensor(
                            out=t1.rearrange("p (k i e) -> p k i e", k=4, i=8),
                            in0=n2.rearrange("p (k i e) -> p k i e", k=4, i=8),
                            in1=ssum3.unsqueeze(2).broadcast_to((128, 4, 8, 8)),
                            op=OP.mult,
                        )
                        nc.vector.tensor_tensor(out=dotp[:], in0=dotp[:], in1=t1[:], op=OP.subtract)
                        ns = sm.tile([128, 256], F32, tag="ns")
                        nc.gpsimd.tensor_tensor(out=ns[:], in0=rstd[:], in1=rstd[:], op=OP.mult)
                        nc.vector.scalar_tensor_tensor(
                            out=ns[:], in0=var[:], scalar=16.0, in1=ns[:],
                            op0=OP.mult, op1=OP.mult,
                        )
                        nc.vector.reciprocal(ns[:], ns[:])
                        nc.vector.tensor_tensor(out=rr[:], in0=dotp[:], in1=ns[:], op=OP.mult)

                    # ---- softmax over i (cols (k, i, n'g)) ----
                    rr4 = rr.rearrange("p (k i e) -> p k i e", k=4, i=8)
                    mx = sm.tile([128, 32], F32, tag="mx")
                    mx3 = mx.rearrange("p (k e) -> p k e", k=4)
                    nc.vector.reduce_max(mx3, rr4.transpose((0, 1, 3, 2)), AX.X)
                    es = sm.tile([128, 256], F32, tag="es")
                    es4 = es.rearrange("p (k i e) -> p k i e", k=4, i=8)
                    nc.vector.tensor_tensor(
                        out=es4, in0=rr4,
                        in1=mx3.unsqueeze(2).broadcast_to((128, 4, 8, 8)),
                        op=OP.subtract,
                    )
                    nc.scalar.activation(es[:], es[:], AF.Exp)
                    Z = sm.tile([128, 32], F32, tag="Z")
                    Z3 = Z.rearrange("p (k e) -> p k e", k=4)
                    nc.vector.reduce_sum(Z3, es4.transpose((0, 1, 3, 2)), AX.X)
                    nc.vector.reciprocal(Z[:], Z[:])
                    sc_b = sm.tile([128, 256], BF16, tag="sc_b")
                    nc.vector.tensor_tensor(
                        out=sc_b.rearrange("p (k i e) -> p k i e", k=4, i=8),
                        in0=es4,
                        in1=Z3.unsqueeze(2).broadcast_to((128, 4, 8, 8)),
                        op=OP.mult,
                    )

                    # ---- s = sum_i sc_i * P (PE accumulate) with correction ----
                    R = big.tile([128, 4096], BF16, tag="Q", name="R")
                    nc.vector.tensor_tensor(
                        out=R.rearrange("p (m d g) -> p m d g", m=64, d=16),
                        in0=P.rearrange("p (m d g) -> p m d g", m=64, d=16),
                        in1=sc_b.rearrange("p (m g) -> p m g", m=64)
                            .unsqueeze(2).broadcast_to((128, 64, 16, 4)),
                        op=OP.mult,
                    )
                    R4 = R.rearrange("p (k i c) -> p k i c", k=4, i=8)
                    s_ps = ps_acc.tile([128, 512], F32, tag="s_ps")
                    s_ps3 = s_ps.rearrange("p (k c) -> p k c", k=4)
                    for i in range(8):
                        nc.tensor.matmul(
                            s_ps3,
                            lhsT=idb_sb[:],
                            rhs=R4[:, :, i],
                            start=(i == 0), stop=(i == 7),
                        )
                    s_sb = sm.tile([128, 512], F32, tag="s_sb")
                    s5 = s_sb.rearrange("p (k n d g) -> p k n d g", k=4, n=2, d=16)
                    if apply_gb:
                        nc.vector.tensor_scalar_mul(out=s_sb[:], in0=s_ps[:], scalar1=1.0)
                    else:
                        t2 = sm.tile([128, 256], F32, tag="t2")
                        nc.gpsimd.tensor_tensor(out=t2[:], in0=sc_b[:], in1=n2[:], op=OP.mult)
                        csc = sm.tile([128, 32], F32, tag="csc")
                        csc3 = csc.rearrange("p (k e) -> p k e", k=4)
                        t2_4 = t2.rearrange("p (k i e) -> p k i e", k=4, i=8)
                        nc.vector.reduce_sum(csc3, t2_4.transpose((0, 1, 3, 2)), AX.X)
                        nc.vector.tensor_tensor(
                            out=s5,
                            in0=s_ps.rearrange("p (k n d g) -> p k n d g", k=4, n=2, d=16),
                            in1=csc.rearrange("p (k n g) -> p k n g", k=4, n=2)
                                .unsqueeze(3).broadcast_to((128, 4, 2, 16, 4)),
                            op=OP.subtract,
                        )

                    # ---- squash over g (cols (k, n', d, g)) ----
                    ssq = sm.tile([128, 512], F32, tag="ssq")
                    nc.scalar.activation(ssq[:], s_sb[:], AF.Square)
                    nsq = sm.tile([128, 128], F32, tag="nsq")
                    nc.vector.reduce_sum(
                        nsq[:], ssq.rearrange("p (m g) -> p m g", m=128), AX.X)
                    sq1 = sm.tile([128, 128], F32, tag="sq1")
                    nc.scalar.activation(sq1[:], nsq[:], AF.Sqrt, bias=eps16[:])
                    nc.vector.scalar_tensor_tensor(
                        out=sq1[:], in0=nsq[:], scalar=1.0, in1=sq1[:],
                        op0=OP.add, op1=OP.mult,
                    )
                    nc.vector.reciprocal(sq1[:], sq1[:])
                    f = sm.tile([128, 128], F32, tag="f")
                    nc.vector.tensor_tensor(out=f[:], in0=nsq[:], in1=sq1[:], op=OP.mult)

                    v_sb = sm.tile([128, 512], F32, tag="v_sb")
                    nc.vector.tensor_tensor(
                        out=v_sb.rearrange("p (m g) -> p m g", m=128),
                        in0=s_sb.rearrange("p (m g) -> p m g", m=128),
                        in1=f[:].unsqueeze(2).broadcast_to((128, 128, 4)),
                        op=OP.mult,
                    )

                    # ---- back to channel-major and stage into v_ch ----
                    vt = ps_acc.tile([128, 512], F32, tag="vt")
                    for k in range(4):
                        nc.tensor.transpose(
                            out=vt[:, k * 128:(k + 1) * 128],
                            in_=v_sb[:, k * 128:(k + 1) * 128],
                            identity=idf_sb[:],
                        )
                    nc.scalar.activation(v_ch[:, sh * 512:(sh + 1) * 512], vt[:], AF.Copy)

                nc.sync.dma_start(out=out.ap()[lb], in_=v_ch[:])

    _split_sync_waits(nc)
    return nc


def _prep_caps(capsules):
    """caps2[h]: [16, 128, XW] bf16; j = lb*8 + i -> image (b=2h+lb, nin=i).
    p 0-63: padded image; p 64-127: same shifted up one padded row."""
    import ml_dtypes
    caps = np.asarray(capsules, np.float32)  # [4, 8, 16, 4, 32, 32]
    B = caps.reshape(4, 8, 64, 32, 32)
    halves = []
    for h in range(2):
        arr = np.zeros((16, 128, XW), np.float32)
        for lb in range(2):
            for i in range(8):
                img = B[2 * h + lb, i]  # [64, 32, 32]
                pad = np.zeros((64, 34, 34), np.float32)
                pad[:, 1:33, 1:33] = img
                flat = pad.reshape(64, 1156)
                j = lb * 8 + i
                arr[j, 0:64, 0:1156] = flat
                arr[j, 64:128, 0:1122] = flat[:, 34:]
        halves.append(np.ascontiguousarray(arr))
    return halves


_CACHE = {}


def kernel(capsules, conv_w, conv_b, ln_gamma, ln_beta):
    import ml_dtypes
    conv_b = np.asarray(conv_b, np.float32)
    ln_gamma = np.asarray(ln_gamma, np.float32)
    ln_beta = np.asarray(ln_beta, np.float32)
    apply_bias = bool(np.any(conv_b))
    apply_gb = bool(np.any(ln_gamma != 1.0) or np.any(ln_beta != 0.0))

    key = (apply_bias, apply_gb)
    if key not in _CACHE:
        _CACHE[key] = build_program(apply_bias=apply_bias, apply_gb=apply_gb)
    nc = _CACHE[key]

    w = np.asarray(conv_w, np.float32)
    wt = np.stack(
        [np.roll(np.rot90(w, k=r, axes=(3, 4)), r, axis=2) for r in range(4)], axis=1
    )
    W512 = np.ascontiguousarray(wt.reshape(512, 64, 3, 3), dtype=np.float32)
    # output-channel order within the pair: m = n'*64 + d*4 + g
    m_idx = np.arange(128)
    m_n = m_idx // 64
    m_d = (m_idx // 4) % 16
    m_g = m_idx % 4
    wpacks = []
    for p in range(4):
        Wc = W512[128 * p: 128 * p + 128]  # [128 outch, 64 inch, 3, 3]
        wp = np.zeros((128, 768), np.float32)
        for kx in range(3):
            wp[0:64, kx * 128:(kx + 1) * 128] = Wc[:, :, 0, kx].T
            wp[64:128, kx * 128:(kx + 1) * 128] = Wc[:, :, 1, kx].T
            wp[0:64, 384 + kx * 128: 384 + (kx + 1) * 128] = Wc[:, :, 2, kx].T
        wpacks.append(np.ascontiguousarray(wp))

    caps_halves = _prep_caps(capsules)

    maskd = np.zeros((128, 8), np.float32)
    maskd[m_idx, m_n * 4 + m_g] = 1.0
    maskd = np.ascontiguousarray(maskd.astype(ml_dtypes.bfloat16))
    identb = np.ascontiguousarray(np.eye(128).astype(ml_dtypes.bfloat16))
    identf = np.eye(128, dtype=np.float32)

    in_maps = []
    for c in range(8):
        p, h = c // 2, c % 2
        m = {
            "caps": caps_halves[h],
            "w": wpacks[p],
            "maskd": maskd,
            "identb": identb,
            "identf": identf,
        }
        if apply_bias:
            cb = conv_b[(2 * p + m_n) * 16 + m_d].reshape(128, 1).astype(np.float32)
            m["cb"] = np.ascontiguousarray(cb)
        if apply_gb:
            m["gam"] = np.ascontiguousarray(ln_gamma.reshape(1, 16))
            m["bet"] = np.ascontiguousarray(ln_beta.reshape(1, 16))
        in_maps.append(m)

    res = run_bass_kernel_spmd(nc, in_maps, core_ids=list(range(8)), trace=False)
    full = np.zeros((4, 8, 16, 4, 32, 32), np.float32)
    for c in range(8):
        p, h = c // 2, c % 2
        o = np.asarray(res.results[c]["out"], np.float32).reshape(2, 2, 16, 4, 32, 32)
        full[2 * h: 2 * h + 2, 2 * p: 2 * p + 2] = o
    return full
